# revision 22
# baseline (speedup 1.0000x reference)
"""MoE routing kernel for Trainium2, expert-parallel across 8 NeuronCores.

Sharding: experts are sorted by dispatch count and dealt round-robin so that
slot position j on every core has the same padded capacity caps[j] (baked into
the SPMD program). The gate/top-k/dispatch runs on host as part of sharding;
each core receives its experts' dispatched token rows in a partition-major
layout (one large contiguous per-partition block per tensor, so every
dma_start moves 4-16KB per partition), its expert weights, and a slice of
tokens for the (replicated-weight) shared expert. Device computes the grouped
SwiGLU expert GEMMs + shared expert with slots always on the moving dim (no
partial-tile matmul waste) and SwiGLU pairs processed two-at-a-time so the
silu/mul chain overlaps the next pair's matmuls. Host gathers per-slot
outputs and does the weighted combine (unshard).
"""

import numpy as np
import ml_dtypes

import bass_rust
import concourse.bass as bass
import concourse.mybir as mybir
from concourse.tile import TileContext
from concourse.vector_clock import ScopedClock
from concourse.bass_utils import run_bass_kernel_spmd

B, T, C = 2, 2048, 2048
N = B * T
E, H, HS = 64, 256, 512
TOPK = 6
NCORES = 8
ELOC = E // NCORES  # 8 experts per core
NLOC = N // NCORES  # 512 tokens per core for the shared expert
BF16 = mybir.dt.bfloat16
F32 = mybir.dt.float32
P = 128
KC = C // P  # 16 contraction chunks over C

_BF16_NP = ml_dtypes.bfloat16

# shared-up pair order: pair i computes y chunk m=i and gate chunk m=4+i of
# the 2*HS=1024 up-projection columns
_MPAIR = [0, 4, 1, 5, 2, 6, 3, 7]


# --------------------------------------------------------------------------
# Tile tail-drain fix: this walrus build allows at most one semaphore wait per
# instruction (none on Drain). Tile's end-of-context drain carries the whole
# global clock; emit a chain of single-wait NOPs on SP instead.
# --------------------------------------------------------------------------
def _patched_drain_and_barrier(self, tick_clock, wait_clock):
    carrier = self.nc.sync.nop(nofuse=True, hint="tail_wait_0")
    wait_clock.add_sem_waits(carrier.ins, ScopedClock({None: tick_clock.global_clock}))
    si = carrier.ins.sync_info
    waits = list(si.on_wait) if si else []
    upds = list(si.on_update) if si else []
    carrier.ins.sync_info = bass_rust.SyncInfo(on_wait=waits[:1], on_update=upds)
    for i, w in enumerate(waits[1:]):
        n2 = self.nc.sync.nop(nofuse=True, hint=f"tail_wait_{i + 1}")
        n2.ins.sync_info = bass_rust.SyncInfo(on_wait=[w], on_update=[])

    self.nc.sync.drain()
    self.nc.all_engine_barrier()
    assert self.sems is not None
    popped = self.nc._tile_sem_poison_stack.pop()
    assert popped is self._sem_poison
    self.nc.clear_and_free_semaphores(list(self.sems.allocated().values()))
    self.nc.all_engine_barrier()


_orig_add_instruction = TileContext._add_instruction


def _patched_add_instruction(self, inst):
    si = getattr(inst, "sync_info", None)
    if si is not None and len(si.on_wait) > 1:
        waits = list(si.on_wait)
        for w in waits[:-1]:
            nop = mybir.InstNoOp(
                name=self.nc.get_next_instruction_name(), ins=[], outs=[])
            nop.engine = inst.engine
            nop.sync_info = bass_rust.SyncInfo(on_wait=[w], on_update=[])
            _orig_add_instruction(self, nop)
        inst.sync_info = bass_rust.SyncInfo(
            on_wait=[waits[-1]], on_update=list(si.on_update))
    _orig_add_instruction(self, inst)


def _install_drain_fix():
    if getattr(TileContext, "_drain_fix_installed", False):
        return
    TileContext._drain_and_barrier = _patched_drain_and_barrier
    TileContext._add_instruction = _patched_add_instruction
    TileContext._drain_fix_installed = True


# --------------------------------------------------------------------------
# Device kernel
# --------------------------------------------------------------------------
_BUILD_CACHE = {}


def _build(caps):
    """Per-core Bass program; caps[j] = padded capacity of slot position j."""
    _install_drain_fix()
    nc = bass.Bass()

    offs = [0]
    for cp in caps:
        offs.append(offs[-1] + cp)
    S = offs[-1]

    # all HBM tensors are partition-major: [128, X] with large contiguous
    # per-partition runs so DMA packets are 4-16KB
    xdh = nc.declare_dram_parameter("xdh", [P, KC * S], BF16, isOutput=False)
    wuh = nc.declare_dram_parameter("wuh", [P, ELOC * KC * 2 * H], BF16, isOutput=False)
    wdh = nc.declare_dram_parameter("wdh", [P, ELOC * 2 * C], BF16, isOutput=False)
    xsh = nc.declare_dram_parameter("xsh", [P, KC * NLOC], BF16, isOutput=False)
    wsuh = nc.declare_dram_parameter("wsuh", [P, KC * 2 * HS], BF16, isOutput=False)
    wsdh = nc.declare_dram_parameter("wsdh", [P, 4 * C], BF16, isOutput=False)
    yrh = nc.declare_dram_parameter("yrh", [P, KC * S], BF16, isOutput=True)
    ysh = nc.declare_dram_parameter("ysh", [NLOC, C], BF16, isOutput=True)

    with TileContext(nc) as tc:
        with (
            tc.tile_pool(name="xsg_sb", bufs=4) as xsg_pool,
            tc.tile_pool(name="wsug0_sb", bufs=2) as wsug0_pool,
            tc.tile_pool(name="wsug_sb", bufs=2) as wsug_pool,
            tc.tile_pool(name="wsd_sb", bufs=1) as wsd_pool,
            tc.tile_pool(name="hsh_sb", bufs=1) as hsh_pool,
            tc.tile_pool(name="osh_sb", bufs=4) as osh_pool,
            tc.tile_pool(name="xd_sb", bufs=2) as xd_pool,
            tc.tile_pool(name="wu_sb", bufs=2) as wu_pool,
            tc.tile_pool(name="wd_sb", bufs=2) as wd_pool,
            tc.tile_pool(name="yo_sb", bufs=2) as yo_pool,
            tc.tile_pool(name="h_sb", bufs=2) as h_pool,
            tc.tile_pool(name="sg_sb", bufs=2) as sg_pool,
            tc.tile_pool(name="pu", bufs=4, space="PSUM") as pu_pool,
            tc.tile_pool(name="pd", bufs=2, space="PSUM") as pd_pool,
        ):
            xd_t = [None] * ELOC
            wu_t = [None] * ELOC
            wd_t = [None] * ELOC
            h_t = [None] * ELOC

            def load_expert_xd(j):
                cap = caps[j]
                base = KC * offs[j]
                xd_t[j] = xd_pool.tile([P, KC * cap], BF16, tag="xd",
                                       name=f"xd_{j}")
                nc.sync.dma_start(
                    out=xd_t[j][:], in_=xdh[:, base:base + KC * cap])

            def load_expert_wu(j):
                wu_t[j] = wu_pool.tile([P, KC * 512], BF16, tag="wu",
                                       name=f"wu_{j}")
                nc.sync.dma_start(
                    out=wu_t[j][:],
                    in_=wuh[:, j * KC * 512:(j + 1) * KC * 512])

            def load_expert_wd(j, engine=None):
                wd_t[j] = wd_pool.tile([P, 2 * C], BF16, tag="wd",
                                       name=f"wd_{j}")
                (engine or nc.gpsimd).dma_start(
                    out=wd_t[j][:], in_=wdh[:, j * 2 * C:(j + 1) * 2 * C])

            def up_expert(j):
                # up in two m-half passes; half hf computes gate chunk m=hf
                # and v chunk m=2+hf -> h chunk hf; half 0's silu/mul
                # overlaps half 1's matmuls
                cap = caps[j]
                xd, wu = xd_t[j], wu_t[j]
                assert cap <= 512
                w_ = cap
                h = h_pool.tile([P, 2 * w_], BF16, tag="h", name=f"h_{j}")
                for hf in range(2):
                    p_g = pu_pool.tile([P, w_], F32, space="PSUM",
                                       tag="pu", name=f"pug_{j}_{hf}")
                    p_v = pu_pool.tile([P, w_], F32, space="PSUM",
                                       tag="pu", name=f"puv_{j}_{hf}")
                    for k in range(KC):
                        lb = k * 512
                        rhs = xd[:, k * cap:k * cap + w_]
                        nc.tensor.matmul(
                            out=p_g[:],
                            lhsT=wu[:, lb + hf * P:lb + (hf + 1) * P],
                            rhs=rhs,
                            start=(k == 0), stop=(k == KC - 1))
                        nc.tensor.matmul(
                            out=p_v[:],
                            lhsT=wu[:, lb + (2 + hf) * P:lb + (3 + hf) * P],
                            rhs=rhs,
                            start=(k == 0), stop=(k == KC - 1))
                    sg = sg_pool.tile([P, w_], F32, tag="sg",
                                      name=f"sg_{j}_{hf}")
                    nc.scalar.activation(sg[:], p_g[:],
                                         mybir.ActivationFunctionType.Silu)
                    nc.vector.tensor_mul(
                        h[:, hf * w_:(hf + 1) * w_], sg[:], p_v[:])
                h_t[j] = h

            def down_expert(j, store_quarters=False):
                # down: out = [128 of C, cap slots]; two C chunks share a
                # 2-bank psum tile (sub-outputs bank-aligned at col 0 / 512)
                cap = caps[j]
                base = KC * offs[j]
                wd, h = wd_t[j], h_t[j]
                w_ = cap
                yo = yo_pool.tile([P, KC * cap], BF16, tag="yo",
                                  name=f"yo_{j}")
                for ccp in range(8):
                    pd = pd_pool.tile([P, 1024], F32, space="PSUM",
                                      tag="pd", name=f"pd_{j}_{ccp}")
                    for sub in range(2):
                        cc = 2 * ccp + sub
                        for kh in range(2):
                            nc.tensor.matmul(
                                out=pd[:, sub * 512:sub * 512 + w_],
                                lhsT=wd[:, kh * C + cc * P:
                                        kh * C + (cc + 1) * P],
                                rhs=h[:, kh * w_:(kh + 1) * w_],
                                start=(kh == 0), stop=(kh == 1))
                    if w_ == 512:
                        dsts = [yo[:, 2 * ccp * cap:(2 * ccp + 2) * cap]]
                        srcs = [pd[:]]
                    else:
                        dsts = [yo[:, (2 * ccp + s) * cap:
                                   (2 * ccp + s) * cap + w_]
                                for s in range(2)]
                        srcs = [pd[:, s * 512:s * 512 + w_] for s in range(2)]
                    for dst, srcp in zip(dsts, srcs):
                        if ccp % 4 == 3:
                            nc.scalar.copy(dst, srcp)
                        else:
                            nc.vector.tensor_copy(out=dst, in_=srcp)
                    if store_quarters:
                        if ccp % 2 == 1:
                            lo = (ccp - 1) * 2 * cap
                            nc.scalar.dma_start(
                                out=yrh[:, base + lo:base + lo + 4 * cap],
                                in_=yo[:, lo:lo + 4 * cap])
                    elif ccp in (3, 7):
                        lo = 0 if ccp == 3 else 8 * cap
                        nc.scalar.dma_start(
                            out=yrh[:, base + lo:base + lo + 8 * cap],
                            in_=yo[:, lo:lo + 8 * cap])

            # ------------- loads: one sync queue, consumption order ---------
            # shared-up pass-0 inputs quartered so the first matmul's inputs
            # land as early as possible
            # first wsu piece is a quarter (k 0-3) so the first real
            # matmul's inputs land as early as possible; remainder in one
            # three-quarter piece
            wsu0_a = wsug0_pool.tile([P, 4 * 256], BF16, tag="wsug0a",
                                     name="wsu_0_a")
            nc.sync.dma_start(out=wsu0_a[:], in_=wsuh[:, 0:1024])
            xs_q = []
            wsu0_b = None
            for qq in range(4):
                t2 = xsg_pool.tile([P, 4 * NLOC], BF16, tag="xsg",
                                   name=f"xs_{qq}")
                nc.sync.dma_start(
                    out=t2[:], in_=xsh[:, qq * 4 * NLOC:(qq + 1) * 4 * NLOC])
                xs_q.append(t2)
                if qq == 0:
                    wsu0_b = wsug0_pool.tile([P, 12 * 256], BF16,
                                             tag="wsug0b", name="wsu_0_b")
                    nc.sync.dma_start(out=wsu0_b[:], in_=wsuh[:, 1024:4096])
            wsu_p = [None] * 4
            for pr in range(1, 4):
                t = wsug_pool.tile([P, KC * 256], BF16, tag="wsug",
                                   name=f"wsu_{pr}")
                nc.sync.dma_start(
                    out=t[:], in_=wsuh[:, pr * 4096:(pr + 1) * 4096])
                wsu_p[pr] = t
            load_expert_xd(0)
            load_expert_wu(0)
            wsd_t = wsd_pool.tile([P, 4 * C], BF16, tag="wsd")
            nc.sync.dma_start(out=wsd_t[:], in_=wsdh[:])
            load_expert_wd(0, engine=nc.sync)
            load_expert_xd(1)
            load_expert_wu(1)
            load_expert_wd(1, engine=nc.sync)

            # ------------- PE warm-up: dummy matmuls with no input deps -----
            # the HAM clock gate starts at 1.2 GHz and needs ~3.4us of
            # sustained PE activity to release; burn the DMA-wait window
            # (scratch = a corner of hsh, whose real writes come much later)
            hsh = hsh_pool.tile([P, 4 * NLOC], BF16, tag="hsh")
            nc.vector.memset(hsh[:, 0:512], 0.0)
            pwarm = pd_pool.tile([P, 1024], F32, space="PSUM", tag="pd",
                                 name="pwarm")
            for wi in range(16):
                nc.tensor.matmul(out=pwarm[:, 0:512], lhsT=hsh[:, 0:P],
                                 rhs=hsh[:, 0:512], start=True, stop=True)

            # ------------- shared up: 4 pair passes x 2 psums ---------------
            # hsh holds h = silu(g_s)*y_s as 4 chunks of [128, 512] columns;
            # each pair's silu/mul overlaps the next pair's matmuls
            for pr in range(4):
                ps_y = pu_pool.tile([P, NLOC], F32, space="PSUM", tag="pu",
                                    name=f"ps_y{pr}")
                ps_g = pu_pool.tile([P, NLOC], F32, space="PSUM", tag="pu",
                                    name=f"ps_g{pr}")
                for k in range(KC):
                    if pr == 0:
                        if k < 4:
                            wt, lo = wsu0_a, k * 256
                        else:
                            wt, lo = wsu0_b, (k - 4) * 256
                    else:
                        wt, lo = wsu_p[pr], k * 256
                    rhs = xs_q[k // 4][:, (k % 4) * NLOC:(k % 4 + 1) * NLOC]
                    nc.tensor.matmul(
                        out=ps_y[:], lhsT=wt[:, lo:lo + P], rhs=rhs,
                        start=(k == 0), stop=(k == KC - 1))
                    nc.tensor.matmul(
                        out=ps_g[:], lhsT=wt[:, lo + P:lo + 2 * P], rhs=rhs,
                        start=(k == 0), stop=(k == KC - 1))
                sg = sg_pool.tile([P, NLOC], F32, tag="sg", name=f"sgs_{pr}")
                nc.scalar.activation(sg[:], ps_g[:],
                                     mybir.ActivationFunctionType.Silu)
                nc.vector.tensor_mul(
                    hsh[:, pr * NLOC:(pr + 1) * NLOC], sg[:], ps_y[:])

            # expert 0's up runs here: it gives the tensor engine work while
            # the shared-down inputs and the expert pipeline loads stream in
            up_expert(0)

            # ------------- shared down: tokens as out rows ------------------
            # two C-column chunks share one 2-bank psum tile; copies all on
            # vector (the scalar ACT copy is ~2x slower and would pace the
            # pd-slot rotation)
            for mt in range(4):
                osh = osh_pool.tile([P, C], BF16, tag="osh", name=f"osh_{mt}")
                for ncp in range(2):
                    pd = pd_pool.tile([P, 1024], F32, space="PSUM", tag="pd",
                                      name=f"pds_{mt}_{ncp}")
                    for sub in range(2):
                        ncc = 2 * ncp + sub
                        for kh in range(4):
                            nc.tensor.matmul(
                                out=pd[:, sub * 512:(sub + 1) * 512],
                                lhsT=hsh[:, kh * NLOC + mt * P:
                                         kh * NLOC + (mt + 1) * P],
                                rhs=wsd_t[:, kh * C + ncc * 512:
                                          kh * C + (ncc + 1) * 512],
                                start=(kh == 0), stop=(kh == 3))
                    nc.vector.tensor_copy(
                        out=osh[:, ncp * 1024:(ncp + 1) * 1024], in_=pd[:])
                nc.gpsimd.dma_start(out=ysh[mt * P:(mt + 1) * P, :], in_=osh[:])

            # ------------- routed experts (software-pipelined) --------------
            # up_{j+1} is issued before down_j so the pd psum->sbuf copies of
            # down_j drain behind up_{j+1}'s matmuls
            up_expert(1)
            down_expert(0)
            for j in range(2, ELOC):
                load_expert_xd(j)
                load_expert_wu(j)
                load_expert_wd(j)
                up_expert(j)
                down_expert(j - 1)
            down_expert(ELOC - 1, store_quarters=True)
    return nc


# --------------------------------------------------------------------------
# Host wrapper
# --------------------------------------------------------------------------
def _pm(a, nchunk):
    """[nchunk*128, X] row-major -> partition-major [128, nchunk*X]."""
    x = a.shape[1]
    return np.ascontiguousarray(
        a.reshape(nchunk, P, x).transpose(1, 0, 2)).reshape(P, nchunk * x)


def kernel(x, w_gate, w_shared_up, w_shared_down, w_up, w_down):
    x_flat = x.reshape(-1, C)

    # ---- gate: sigmoid scores, top-6, normalized weights (f64 for a stable
    # ordering; ties in the fp32 reference are measure-zero) ----
    logits = x_flat.astype(np.float64) @ w_gate.astype(np.float64)
    scores = 1.0 / (1.0 + np.exp(-logits))
    topk_idx = np.argsort(-scores, axis=-1, kind="stable")[:, :TOPK]
    w = np.take_along_axis(scores, topk_idx, axis=-1)
    w = w / w.sum(-1, keepdims=True)

    # ---- dispatch positions (stable within each expert, slot-major order) --
    flat_e = topk_idx.reshape(-1)
    order = np.argsort(flat_e, kind="stable")
    sorted_e = flat_e[order]
    group_start = np.searchsorted(sorted_e, np.arange(E))
    counts = np.bincount(flat_e, minlength=E)

    token_of_slot = np.arange(N * TOPK) // TOPK
    expert_slots = []   # flat (token,k) slot ids, dispatch order, per expert
    expert_tokens = []
    for e in range(E):
        slots = order[group_start[e]: group_start[e] + counts[e]]
        expert_slots.append(slots)
        expert_tokens.append(token_of_slot[slots])

    # ---- balanced expert->core assignment: sort by count desc, deal 8 at a
    # time; slot position j has the same padded cap on every core ----
    ranks = np.argsort(-counts, kind="stable")
    expert_of = [[int(ranks[8 * j + c]) for j in range(ELOC)]
                 for c in range(NCORES)]
    caps = tuple(
        max(8, int(-(-int(counts[ranks[8 * j]]) // 8) * 8)) for j in range(ELOC))
    offs = [0]
    for cp in caps:
        offs.append(offs[-1] + cp)

    # ---- build per-core inputs (partition-major bf16) ----
    xT_bf = np.ascontiguousarray(x_flat.T).astype(_BF16_NP)  # [C, N]
    wsu_f = w_shared_up.astype(_BF16_NP)
    wsd_f = w_shared_down.astype(_BF16_NP)

    # shared-up weights in pair/k-major order (see _MPAIR)
    wr = wsu_f.reshape(KC, P, 8, P)[:, :, _MPAIR, :]        # [k, p, 8, 128]
    wsuh = np.ascontiguousarray(
        wr.reshape(KC, P, 4, 2 * P).transpose(1, 2, 0, 3)).reshape(P, KC * 2 * HS)
    wsdh = _pm(wsd_f, 4)

    in_maps = []
    for c in range(NCORES):
        xd_blocks = []
        wu_blocks = []
        wd_blocks = []
        for j in range(ELOC):
            e = expert_of[c][j]
            tok = expert_tokens[e]
            n = len(tok)
            blk = np.zeros((P, KC, caps[j]), dtype=_BF16_NP)
            blk[:, :, :n] = xT_bf[:, tok].reshape(KC, P, n).transpose(1, 0, 2)
            xd_blocks.append(blk.reshape(P, -1))
            wu_blocks.append(_pm(w_up[e].astype(_BF16_NP), KC))
            wd_blocks.append(_pm(w_down[e].astype(_BF16_NP), 2))
        xsh = _pm(np.ascontiguousarray(
            xT_bf[:, c * NLOC:(c + 1) * NLOC]), KC)
        in_maps.append({
            "xdh": np.concatenate(xd_blocks, axis=1),
            "wuh": np.concatenate(wu_blocks, axis=1),
            "wdh": np.concatenate(wd_blocks, axis=1),
            "xsh": xsh,
            "wsuh": wsuh,
            "wsdh": wsdh,
        })

    if caps not in _BUILD_CACHE:
        _BUILD_CACHE[caps] = _build(caps)
    nc = _BUILD_CACHE[caps]

    res = run_bass_kernel_spmd(nc, in_maps, list(range(NCORES)))
    if res.exec_time_ns is not None:
        print(f"HW exec time: {res.exec_time_ns} ns", flush=True)

    # ---- host combine (unshard): gather per-slot rows, weight, sum ----
    y_ts = np.empty((N * TOPK, C), dtype=np.float32)
    for c in range(NCORES):
        yr = res.results[c]["yrh"]
        for j in range(ELOC):
            e = expert_of[c][j]
            n = int(counts[e])
            seg = yr[:, KC * offs[j]:KC * offs[j] + KC * caps[j]]
            seg = seg.reshape(P, KC, caps[j])[:, :, :n]
            y_ts[expert_slots[e]] = (
                seg.transpose(2, 1, 0).reshape(n, C).astype(np.float32))
    routed = (y_ts.reshape(N, TOPK, C)
              * w.reshape(N, TOPK, 1).astype(np.float32)).sum(axis=1)
    shared = np.concatenate(
        [r["ysh"] for r in res.results], axis=0).astype(np.float32)
    return (shared + routed).reshape(B, T, C).astype(np.float32)


# revision 23
# speedup vs baseline: 1.0159x; 1.0159x over previous
"""MoE routing kernel for Trainium2, expert-parallel across 8 NeuronCores.

Sharding: experts are sorted by dispatch count and dealt round-robin so that
slot position j on every core has the same padded capacity caps[j] (baked into
the SPMD program). The gate/top-k/dispatch runs on host as part of sharding;
each core receives its experts' dispatched token rows in a partition-major
layout (one large contiguous per-partition block per tensor, so every
dma_start moves 4-16KB per partition), its expert weights, and a slice of
tokens for the (replicated-weight) shared expert. Device computes the grouped
SwiGLU expert GEMMs + shared expert with slots always on the moving dim (no
partial-tile matmul waste) and SwiGLU pairs processed two-at-a-time so the
silu/mul chain overlaps the next pair's matmuls. Host gathers per-slot
outputs and does the weighted combine (unshard).
"""

import numpy as np
import ml_dtypes

import bass_rust
import concourse.bass as bass
import concourse.mybir as mybir
from concourse.tile import TileContext
from concourse.vector_clock import ScopedClock
from concourse.bass_utils import run_bass_kernel_spmd

B, T, C = 2, 2048, 2048
N = B * T
E, H, HS = 64, 256, 512
TOPK = 6
NCORES = 8
ELOC = E // NCORES  # 8 experts per core
NLOC = N // NCORES  # 512 tokens per core for the shared expert
BF16 = mybir.dt.bfloat16
F32 = mybir.dt.float32
P = 128
KC = C // P  # 16 contraction chunks over C

_BF16_NP = ml_dtypes.bfloat16

# shared-up pair order: pair i computes y chunk m=i and gate chunk m=4+i of
# the 2*HS=1024 up-projection columns
_MPAIR = [0, 4, 1, 5, 2, 6, 3, 7]


# --------------------------------------------------------------------------
# Tile tail-drain fix: this walrus build allows at most one semaphore wait per
# instruction (none on Drain). Tile's end-of-context drain carries the whole
# global clock; emit a chain of single-wait NOPs on SP instead.
# --------------------------------------------------------------------------
def _patched_drain_and_barrier(self, tick_clock, wait_clock):
    carrier = self.nc.sync.nop(nofuse=True, hint="tail_wait_0")
    wait_clock.add_sem_waits(carrier.ins, ScopedClock({None: tick_clock.global_clock}))
    si = carrier.ins.sync_info
    waits = list(si.on_wait) if si else []
    upds = list(si.on_update) if si else []
    carrier.ins.sync_info = bass_rust.SyncInfo(on_wait=waits[:1], on_update=upds)
    for i, w in enumerate(waits[1:]):
        n2 = self.nc.sync.nop(nofuse=True, hint=f"tail_wait_{i + 1}")
        n2.ins.sync_info = bass_rust.SyncInfo(on_wait=[w], on_update=[])

    self.nc.sync.drain()
    self.nc.all_engine_barrier()
    assert self.sems is not None
    popped = self.nc._tile_sem_poison_stack.pop()
    assert popped is self._sem_poison
    self.nc.clear_and_free_semaphores(list(self.sems.allocated().values()))
    self.nc.all_engine_barrier()


_orig_add_instruction = TileContext._add_instruction


def _patched_add_instruction(self, inst):
    si = getattr(inst, "sync_info", None)
    if si is not None and len(si.on_wait) > 1:
        waits = list(si.on_wait)
        for w in waits[:-1]:
            nop = mybir.InstNoOp(
                name=self.nc.get_next_instruction_name(), ins=[], outs=[])
            nop.engine = inst.engine
            nop.sync_info = bass_rust.SyncInfo(on_wait=[w], on_update=[])
            _orig_add_instruction(self, nop)
        inst.sync_info = bass_rust.SyncInfo(
            on_wait=[waits[-1]], on_update=list(si.on_update))
    _orig_add_instruction(self, inst)


def _install_drain_fix():
    if getattr(TileContext, "_drain_fix_installed", False):
        return
    TileContext._drain_and_barrier = _patched_drain_and_barrier
    TileContext._add_instruction = _patched_add_instruction
    TileContext._drain_fix_installed = True


# --------------------------------------------------------------------------
# Device kernel
# --------------------------------------------------------------------------
_BUILD_CACHE = {}


def _build(caps):
    """Per-core Bass program; caps[j] = padded capacity of slot position j."""
    _install_drain_fix()
    nc = bass.Bass()

    offs = [0]
    for cp in caps:
        offs.append(offs[-1] + cp)
    S = offs[-1]

    # all HBM tensors are partition-major: [128, X] with large contiguous
    # per-partition runs so DMA packets are 4-16KB
    # xd and wu interleaved per expert into one tensor -> one DMA per expert
    xwh = nc.declare_dram_parameter(
        "xwh", [P, KC * S + ELOC * KC * 2 * H], BF16, isOutput=False)
    wdh = nc.declare_dram_parameter("wdh", [P, ELOC * 2 * C], BF16, isOutput=False)
    xsh = nc.declare_dram_parameter("xsh", [P, KC * NLOC], BF16, isOutput=False)
    wsuh = nc.declare_dram_parameter("wsuh", [P, KC * 2 * HS], BF16, isOutput=False)
    wsdh = nc.declare_dram_parameter("wsdh", [P, 4 * C], BF16, isOutput=False)
    yrh = nc.declare_dram_parameter("yrh", [P, KC * S], BF16, isOutput=True)
    ysh = nc.declare_dram_parameter("ysh", [NLOC, C], BF16, isOutput=True)

    with TileContext(nc) as tc:
        with (
            tc.tile_pool(name="xsg_sb", bufs=4) as xsg_pool,
            tc.tile_pool(name="wsug0_sb", bufs=2) as wsug0_pool,
            tc.tile_pool(name="wsug_sb", bufs=2) as wsug_pool,
            tc.tile_pool(name="wsd_sb", bufs=1) as wsd_pool,
            tc.tile_pool(name="hsh_sb", bufs=1) as hsh_pool,
            tc.tile_pool(name="osh_sb", bufs=4) as osh_pool,
            tc.tile_pool(name="xw_sb", bufs=2) as xw_pool,
            tc.tile_pool(name="wd_sb", bufs=2) as wd_pool,
            tc.tile_pool(name="yo_sb", bufs=2) as yo_pool,
            tc.tile_pool(name="h_sb", bufs=2) as h_pool,
            tc.tile_pool(name="sg_sb", bufs=2) as sg_pool,
            tc.tile_pool(name="pu", bufs=4, space="PSUM") as pu_pool,
            tc.tile_pool(name="pd", bufs=2, space="PSUM") as pd_pool,
        ):
            xw_t = [None] * ELOC
            wd_t = [None] * ELOC
            h_t = [None] * ELOC

            def load_expert_xw(j):
                cap = caps[j]
                base = KC * offs[j] + j * KC * 512
                n = KC * cap + KC * 512
                xw_t[j] = xw_pool.tile([P, n], BF16, tag="xw",
                                       name=f"xw_{j}")
                nc.sync.dma_start(
                    out=xw_t[j][:], in_=xwh[:, base:base + n])

            def load_expert_wd(j, engine=None):
                wd_t[j] = wd_pool.tile([P, 2 * C], BF16, tag="wd",
                                       name=f"wd_{j}")
                (engine or nc.gpsimd).dma_start(
                    out=wd_t[j][:], in_=wdh[:, j * 2 * C:(j + 1) * 2 * C])

            def up_expert(j):
                # up in two m-half passes; half hf computes gate chunk m=hf
                # and v chunk m=2+hf -> h chunk hf; half 0's silu/mul
                # overlaps half 1's matmuls
                cap = caps[j]
                xw = xw_t[j]
                wuo = KC * cap   # wu columns start after the xd block
                assert cap <= 512
                w_ = cap
                h = h_pool.tile([P, 2 * w_], BF16, tag="h", name=f"h_{j}")
                for hf in range(2):
                    p_g = pu_pool.tile([P, w_], F32, space="PSUM",
                                       tag="pu", name=f"pug_{j}_{hf}")
                    p_v = pu_pool.tile([P, w_], F32, space="PSUM",
                                       tag="pu", name=f"puv_{j}_{hf}")
                    for k in range(KC):
                        lb = wuo + k * 512
                        rhs = xw[:, k * cap:k * cap + w_]
                        nc.tensor.matmul(
                            out=p_g[:],
                            lhsT=xw[:, lb + hf * P:lb + (hf + 1) * P],
                            rhs=rhs,
                            start=(k == 0), stop=(k == KC - 1))
                        nc.tensor.matmul(
                            out=p_v[:],
                            lhsT=xw[:, lb + (2 + hf) * P:lb + (3 + hf) * P],
                            rhs=rhs,
                            start=(k == 0), stop=(k == KC - 1))
                    sg = sg_pool.tile([P, w_], F32, tag="sg",
                                      name=f"sg_{j}_{hf}")
                    nc.scalar.activation(sg[:], p_g[:],
                                         mybir.ActivationFunctionType.Silu)
                    nc.vector.tensor_mul(
                        h[:, hf * w_:(hf + 1) * w_], sg[:], p_v[:])
                h_t[j] = h

            def down_expert(j, store_quarters=False):
                # down: out = [128 of C, cap slots]; two C chunks share a
                # 2-bank psum tile (sub-outputs bank-aligned at col 0 / 512)
                cap = caps[j]
                base = KC * offs[j]
                wd, h = wd_t[j], h_t[j]
                w_ = cap
                yo = yo_pool.tile([P, KC * cap], BF16, tag="yo",
                                  name=f"yo_{j}")
                for ccp in range(8):
                    pd = pd_pool.tile([P, 1024], F32, space="PSUM",
                                      tag="pd", name=f"pd_{j}_{ccp}")
                    for sub in range(2):
                        cc = 2 * ccp + sub
                        for kh in range(2):
                            nc.tensor.matmul(
                                out=pd[:, sub * 512:sub * 512 + w_],
                                lhsT=wd[:, kh * C + cc * P:
                                        kh * C + (cc + 1) * P],
                                rhs=h[:, kh * w_:(kh + 1) * w_],
                                start=(kh == 0), stop=(kh == 1))
                    if w_ == 512:
                        dsts = [yo[:, 2 * ccp * cap:(2 * ccp + 2) * cap]]
                        srcs = [pd[:]]
                    else:
                        dsts = [yo[:, (2 * ccp + s) * cap:
                                   (2 * ccp + s) * cap + w_]
                                for s in range(2)]
                        srcs = [pd[:, s * 512:s * 512 + w_] for s in range(2)]
                    for dst, srcp in zip(dsts, srcs):
                        if ccp % 4 == 3:
                            nc.scalar.copy(dst, srcp)
                        else:
                            nc.vector.tensor_copy(out=dst, in_=srcp)
                    if store_quarters:
                        if ccp % 2 == 1:
                            lo = (ccp - 1) * 2 * cap
                            nc.scalar.dma_start(
                                out=yrh[:, base + lo:base + lo + 4 * cap],
                                in_=yo[:, lo:lo + 4 * cap])
                    elif ccp in (3, 7):
                        lo = 0 if ccp == 3 else 8 * cap
                        nc.scalar.dma_start(
                            out=yrh[:, base + lo:base + lo + 8 * cap],
                            in_=yo[:, lo:lo + 8 * cap])

            # ------------- loads: one sync queue, consumption order ---------
            # shared-up pass-0 inputs quartered so the first matmul's inputs
            # land as early as possible
            # first wsu piece is a quarter (k 0-3) so the first real
            # matmul's inputs land as early as possible; remainder in one
            # three-quarter piece
            wsu0_a = wsug0_pool.tile([P, 4 * 256], BF16, tag="wsug0a",
                                     name="wsu_0_a")
            nc.sync.dma_start(out=wsu0_a[:], in_=wsuh[:, 0:1024])
            xs_q = []
            wsu0_b = None
            for qq in range(4):
                t2 = xsg_pool.tile([P, 4 * NLOC], BF16, tag="xsg",
                                   name=f"xs_{qq}")
                nc.sync.dma_start(
                    out=t2[:], in_=xsh[:, qq * 4 * NLOC:(qq + 1) * 4 * NLOC])
                xs_q.append(t2)
                if qq == 0:
                    wsu0_b = wsug0_pool.tile([P, 12 * 256], BF16,
                                             tag="wsug0b", name="wsu_0_b")
                    nc.sync.dma_start(out=wsu0_b[:], in_=wsuh[:, 1024:4096])
            wsu_p = [None] * 4
            for pr in range(1, 4):
                t = wsug_pool.tile([P, KC * 256], BF16, tag="wsug",
                                   name=f"wsu_{pr}")
                nc.sync.dma_start(
                    out=t[:], in_=wsuh[:, pr * 4096:(pr + 1) * 4096])
                wsu_p[pr] = t
            load_expert_xw(0)
            wsd_t = wsd_pool.tile([P, 4 * C], BF16, tag="wsd")
            nc.sync.dma_start(out=wsd_t[:], in_=wsdh[:])
            load_expert_wd(0, engine=nc.sync)
            load_expert_xw(1)
            load_expert_wd(1, engine=nc.sync)

            # ------------- PE warm-up: dummy matmuls with no input deps -----
            # the HAM clock gate starts at 1.2 GHz and needs ~3.4us of
            # sustained PE activity to release; burn the DMA-wait window
            # (scratch = a corner of hsh, whose real writes come much later)
            hsh = hsh_pool.tile([P, 4 * NLOC], BF16, tag="hsh")
            nc.vector.memset(hsh[:, 0:512], 0.0)
            pwarm = pd_pool.tile([P, 1024], F32, space="PSUM", tag="pd",
                                 name="pwarm")
            for wi in range(16):
                nc.tensor.matmul(out=pwarm[:, 0:512], lhsT=hsh[:, 0:P],
                                 rhs=hsh[:, 0:512], start=True, stop=True)

            # ------------- shared up: 4 pair passes x 2 psums ---------------
            # hsh holds h = silu(g_s)*y_s as 4 chunks of [128, 512] columns;
            # each pair's silu/mul overlaps the next pair's matmuls
            for pr in range(4):
                ps_y = pu_pool.tile([P, NLOC], F32, space="PSUM", tag="pu",
                                    name=f"ps_y{pr}")
                ps_g = pu_pool.tile([P, NLOC], F32, space="PSUM", tag="pu",
                                    name=f"ps_g{pr}")
                for k in range(KC):
                    if pr == 0:
                        if k < 4:
                            wt, lo = wsu0_a, k * 256
                        else:
                            wt, lo = wsu0_b, (k - 4) * 256
                    else:
                        wt, lo = wsu_p[pr], k * 256
                    rhs = xs_q[k // 4][:, (k % 4) * NLOC:(k % 4 + 1) * NLOC]
                    nc.tensor.matmul(
                        out=ps_y[:], lhsT=wt[:, lo:lo + P], rhs=rhs,
                        start=(k == 0), stop=(k == KC - 1))
                    nc.tensor.matmul(
                        out=ps_g[:], lhsT=wt[:, lo + P:lo + 2 * P], rhs=rhs,
                        start=(k == 0), stop=(k == KC - 1))
                sg = sg_pool.tile([P, NLOC], F32, tag="sg", name=f"sgs_{pr}")
                nc.scalar.activation(sg[:], ps_g[:],
                                     mybir.ActivationFunctionType.Silu)
                nc.vector.tensor_mul(
                    hsh[:, pr * NLOC:(pr + 1) * NLOC], sg[:], ps_y[:])

            # expert 0's up runs here: it gives the tensor engine work while
            # the shared-down inputs and the expert pipeline loads stream in
            up_expert(0)

            # ------------- shared down: tokens as out rows ------------------
            # two C-column chunks share one 2-bank psum tile; copies all on
            # vector (the scalar ACT copy is ~2x slower and would pace the
            # pd-slot rotation)
            for mt in range(4):
                osh = osh_pool.tile([P, C], BF16, tag="osh", name=f"osh_{mt}")
                for ncp in range(2):
                    pd = pd_pool.tile([P, 1024], F32, space="PSUM", tag="pd",
                                      name=f"pds_{mt}_{ncp}")
                    for sub in range(2):
                        ncc = 2 * ncp + sub
                        for kh in range(4):
                            nc.tensor.matmul(
                                out=pd[:, sub * 512:(sub + 1) * 512],
                                lhsT=hsh[:, kh * NLOC + mt * P:
                                         kh * NLOC + (mt + 1) * P],
                                rhs=wsd_t[:, kh * C + ncc * 512:
                                          kh * C + (ncc + 1) * 512],
                                start=(kh == 0), stop=(kh == 3))
                    nc.vector.tensor_copy(
                        out=osh[:, ncp * 1024:(ncp + 1) * 1024], in_=pd[:])
                nc.gpsimd.dma_start(out=ysh[mt * P:(mt + 1) * P, :], in_=osh[:])

            # ------------- routed experts (software-pipelined) --------------
            # up_{j+1} is issued before down_j so the pd psum->sbuf copies of
            # down_j drain behind up_{j+1}'s matmuls
            up_expert(1)
            down_expert(0)
            for j in range(2, ELOC):
                load_expert_xw(j)
                load_expert_wd(j)
                up_expert(j)
                down_expert(j - 1)
            down_expert(ELOC - 1, store_quarters=True)
    return nc


# --------------------------------------------------------------------------
# Host wrapper
# --------------------------------------------------------------------------
def _pm(a, nchunk):
    """[nchunk*128, X] row-major -> partition-major [128, nchunk*X]."""
    x = a.shape[1]
    return np.ascontiguousarray(
        a.reshape(nchunk, P, x).transpose(1, 0, 2)).reshape(P, nchunk * x)


def kernel(x, w_gate, w_shared_up, w_shared_down, w_up, w_down):
    x_flat = x.reshape(-1, C)

    # ---- gate: sigmoid scores, top-6, normalized weights (f64 for a stable
    # ordering; ties in the fp32 reference are measure-zero) ----
    logits = x_flat.astype(np.float64) @ w_gate.astype(np.float64)
    scores = 1.0 / (1.0 + np.exp(-logits))
    topk_idx = np.argsort(-scores, axis=-1, kind="stable")[:, :TOPK]
    w = np.take_along_axis(scores, topk_idx, axis=-1)
    w = w / w.sum(-1, keepdims=True)

    # ---- dispatch positions (stable within each expert, slot-major order) --
    flat_e = topk_idx.reshape(-1)
    order = np.argsort(flat_e, kind="stable")
    sorted_e = flat_e[order]
    group_start = np.searchsorted(sorted_e, np.arange(E))
    counts = np.bincount(flat_e, minlength=E)

    token_of_slot = np.arange(N * TOPK) // TOPK
    expert_slots = []   # flat (token,k) slot ids, dispatch order, per expert
    expert_tokens = []
    for e in range(E):
        slots = order[group_start[e]: group_start[e] + counts[e]]
        expert_slots.append(slots)
        expert_tokens.append(token_of_slot[slots])

    # ---- balanced expert->core assignment: sort by count desc, deal 8 at a
    # time; slot position j has the same padded cap on every core ----
    ranks = np.argsort(-counts, kind="stable")
    expert_of = [[int(ranks[8 * j + c]) for j in range(ELOC)]
                 for c in range(NCORES)]
    caps = tuple(
        max(8, int(-(-int(counts[ranks[8 * j]]) // 8) * 8)) for j in range(ELOC))
    offs = [0]
    for cp in caps:
        offs.append(offs[-1] + cp)

    # ---- build per-core inputs (partition-major bf16) ----
    xT_bf = np.ascontiguousarray(x_flat.T).astype(_BF16_NP)  # [C, N]
    wsu_f = w_shared_up.astype(_BF16_NP)
    wsd_f = w_shared_down.astype(_BF16_NP)

    # shared-up weights in pair/k-major order (see _MPAIR)
    wr = wsu_f.reshape(KC, P, 8, P)[:, :, _MPAIR, :]        # [k, p, 8, 128]
    wsuh = np.ascontiguousarray(
        wr.reshape(KC, P, 4, 2 * P).transpose(1, 2, 0, 3)).reshape(P, KC * 2 * HS)
    wsdh = _pm(wsd_f, 4)

    in_maps = []
    for c in range(NCORES):
        xw_blocks = []
        wd_blocks = []
        for j in range(ELOC):
            e = expert_of[c][j]
            tok = expert_tokens[e]
            n = len(tok)
            blk = np.zeros((P, KC, caps[j]), dtype=_BF16_NP)
            blk[:, :, :n] = xT_bf[:, tok].reshape(KC, P, n).transpose(1, 0, 2)
            xw_blocks.append(blk.reshape(P, -1))
            xw_blocks.append(_pm(w_up[e].astype(_BF16_NP), KC))
            wd_blocks.append(_pm(w_down[e].astype(_BF16_NP), 2))
        xsh = _pm(np.ascontiguousarray(
            xT_bf[:, c * NLOC:(c + 1) * NLOC]), KC)
        in_maps.append({
            "xwh": np.concatenate(xw_blocks, axis=1),
            "wdh": np.concatenate(wd_blocks, axis=1),
            "xsh": xsh,
            "wsuh": wsuh,
            "wsdh": wsdh,
        })

    if caps not in _BUILD_CACHE:
        _BUILD_CACHE[caps] = _build(caps)
    nc = _BUILD_CACHE[caps]

    res = run_bass_kernel_spmd(nc, in_maps, list(range(NCORES)))
    if res.exec_time_ns is not None:
        print(f"HW exec time: {res.exec_time_ns} ns", flush=True)

    # ---- host combine (unshard): gather per-slot rows, weight, sum ----
    y_ts = np.empty((N * TOPK, C), dtype=np.float32)
    for c in range(NCORES):
        yr = res.results[c]["yrh"]
        for j in range(ELOC):
            e = expert_of[c][j]
            n = int(counts[e])
            seg = yr[:, KC * offs[j]:KC * offs[j] + KC * caps[j]]
            seg = seg.reshape(P, KC, caps[j])[:, :, :n]
            y_ts[expert_slots[e]] = (
                seg.transpose(2, 1, 0).reshape(n, C).astype(np.float32))
    routed = (y_ts.reshape(N, TOPK, C)
              * w.reshape(N, TOPK, 1).astype(np.float32)).sum(axis=1)
    shared = np.concatenate(
        [r["ysh"] for r in res.results], axis=0).astype(np.float32)
    return (shared + routed).reshape(B, T, C).astype(np.float32)
